# revision 21
# baseline (speedup 1.0000x reference)
"""Multi-head attention (B=2, S=2048, D=768, H=12) on 8 TRN2 NeuronCores.

Sharding: data-parallel over batch (2) x tensor-parallel over heads
(4 groups of 3 heads), Megatron-style. Core c handles batch c//4 and
heads 3*(c%4) .. 3*(c%4)+2. Each core computes a partial (S, D) output
(its heads' contribution through Wo); the host sums the 4 partials per
batch. bo is added on exactly one core per batch (the others get zeros).

Device kernel (per core), all matmuls bf16 with fp32 PSUM accumulation,
every matmul padded to M=128 output partitions (keeps FWL + PE activity
monitor engaged):
  phase 1: qT/kT (head-dim on partitions, zero-padded chunks for head 2)
           and v (natural layout, ones column at 64, zero-padded to 128
           for the softmax denominator) projected from xT = x[b].T.
  phase 2: per 1024-wide q block, per head: scoresT = k @ qT on PE
           (h0/h1 row-packed via K=64 tile positions), exp(scores/8) on
           ACT (PSUM->SBUF, bf16), outT = [v|1|0]^T @ attnT accumulated
           over 16 k-tiles in PSUM; row 64 of the accumulator is the
           softmax denominator. Denominators go: DVE copy (partition 64)
           -> SBUF->SBUF DMA to partition 0 -> DVE reciprocal -> GPSIMD
           partition_broadcast to a (64, 1024) tile -> the outT copy is
           a fused normalize (tensor_tensor mult).
  phase 3: per 128-row output block: P = sum_h outT_h.T @ Wo_h in one
           PSUM accumulation group, one DVE add of bo, DMA out. Shares
           the accumulator PSUM slots so it overlaps the next q block.
"""

import numpy as np
import ml_dtypes

BF16 = ml_dtypes.bfloat16

B, S, D = 2, 2048, 768
H, HD = 12, 64
HPC = 3            # heads per core
DC = HPC * HD      # 192 projection columns per core
NKT = S // 128     # 16 k-tiles
NDT = D // 128     # 6 contraction tiles for projections
QB = 1024          # q-block width for scores/exp
NQB = S // QB      # 2

_cache = {}


def _patch_ldw_opt():
    # walrus is invoked with --enable-ldw-opt=false by default; redundant
    # LDWEIGHTS elision is a large win for back-to-back matmul streams.
    import concourse.bass_utils as bu
    if getattr(bu, "_ldw_patched", False):
        return
    orig = bu.run_command

    def patched(argv, **kw):
        argv = [a
                for a in argv]
        return orig(argv, **kw)

    bu.run_command = patched
    bu._ldw_patched = True


def _build_nc():
    _patch_ldw_opt()
    import concourse.bacc as bacc
    import concourse.mybir as mybir
    import concourse.tile as tile

    f32 = mybir.dt.float32
    bf16 = mybir.dt.bfloat16
    Exp = mybir.ActivationFunctionType.Exp

    nc = bacc.Bacc("TRN2", target_bir_lowering=False, debug=False, num_devices=1)

    def mm(out_ap, lhsT, rhs, start, stop, nmax=512):
        n = rhs.shape[-1]
        for i in range(0, n, nmax):
            j = min(i + nmax, n)
            nc.tensor.matmul(out_ap[:, i:j], lhsT, rhs[:, i:j],
                             start=start, stop=stop)

    # wqkv columns: [q01 | q2+pad | k01 | k2+pad | v]
    xT = nc.dram_tensor("xT", (D, S), bf16, kind="ExternalInput")
    wqkv = nc.dram_tensor("wqkv", (D, 4 * 128 + DC), bf16, kind="ExternalInput")
    wo = nc.dram_tensor("wo", (HPC * HD, D), bf16, kind="ExternalInput")
    bqk0 = nc.dram_tensor("bqk0", (128, 2), f32, kind="ExternalInput")
    bqk1 = nc.dram_tensor("bqk1", (128, 2), f32, kind="ExternalInput")
    bv = nc.dram_tensor("bv", (1, DC), bf16, kind="ExternalInput")
    bo_t = nc.dram_tensor("bo_t", (128, D), f32, kind="ExternalInput")
    out = nc.dram_tensor("out", (S, D), f32, kind="ExternalOutput")

    with tile.TileContext(nc) as tc:
        with (
            tc.tile_pool(name="persist", bufs=1) as sbp,
            tc.tile_pool(name="att", bufs=6) as att,
            tc.tile_pool(name="stagp", bufs=2) as stagp,
            tc.tile_pool(name="dbcp", bufs=3) as dbcp,
            tc.tile_pool(name="orwp", bufs=4) as orwp,
            tc.tile_pool(name="accsb", bufs=9) as accsb,
            tc.tile_pool(name="scp", bufs=2, space="PSUM") as scp,
            tc.tile_pool(name="acp", bufs=1, space="PSUM") as acp,
            tc.tile_pool(name="pjp", bufs=2, space="PSUM") as pjp,
        ):
            # ---- persistent SBUF tensors + input DMAs ----
            xT_sb = []
            wqkv_sb = []
            for kt in range(NDT):
                ks = slice(kt * 128, (kt + 1) * 128)
                xt = sbp.tile([128, S], bf16, name=f"xT_sb{kt}")
                nc.sync.dma_start(out=xt, in_=xT.ap()[ks, :])
                xT_sb.append(xt)
                wt = sbp.tile([128, 4 * 128 + DC], bf16, name=f"wqkv_sb{kt}")
                nc.sync.dma_start(out=wt, in_=wqkv.ap()[ks, :])
                wqkv_sb.append(wt)
            wo_ab = sbp.tile([128, D], bf16)     # Wo rows for h0 (p0:64) / h1
            nc.sync.dma_start(out=wo_ab, in_=wo.ap()[0:128, :])
            wo_c2 = sbp.tile([HD, D], bf16)      # Wo rows for h2
            nc.sync.dma_start(out=wo_c2, in_=wo.ap()[128:192, :])
            bqk0_sb = sbp.tile([128, 2], f32)
            nc.sync.dma_start(out=bqk0_sb, in_=bqk0.ap())
            bqk1_sb = sbp.tile([128, 2], f32)
            nc.sync.dma_start(out=bqk1_sb, in_=bqk1.ap())
            bv_sb = sbp.tile([1, DC], bf16)
            nc.sync.dma_start(out=bv_sb, in_=bv.ap())
            bo_sb = sbp.tile([128, D], f32)
            nc.sync.dma_start(out=bo_sb, in_=bo_t.ap())

            ones_row = sbp.tile([1, 128], bf16)
            nc.vector.memset(ones_row, 1.0)
            ones_col = sbp.tile([128, 1], bf16)
            nc.vector.memset(ones_col, 1.0)

            # warm up the ACT exp table early (overlaps the input DMAs)
            wu = sbp.tile([1, 8], f32)
            nc.vector.memset(wu, 0.0)
            wu2 = sbp.tile([1, 8], f32)
            nc.scalar.activation(wu2, wu, Exp, scale=1.0)

            qT0 = sbp.tile([128, S], bf16)   # heads 0 (p0:64) / 1 (p64:128)
            kT0 = sbp.tile([128, S], bf16)
            qT1 = sbp.tile([128, S], bf16)   # head 2, duplicated on both halves
            kT1 = sbp.tile([128, S], bf16)
            vv = [sbp.tile([128, HPC, HD], bf16, name=f"vv{st}")
                  for st in range(NKT)]
            outT_ab = sbp.tile([128, S], bf16)   # normalized outT h0/h1 stacked
            outT_c = sbp.tile([HD, S], bf16)     # normalized outT h2
            drow = sbp.tile([1, HPC * S], f32)
            drec = sbp.tile([1, HPC * S], f32)

            # ---- phase 1: projections ----
            proj = (
                ("kc0", 0, 128, kT0, bqk0_sb, 1),
                ("kc1", 128, 128, kT1, bqk1_sb, 1),
                ("qc0", 256, 128, qT0, bqk0_sb, 0),
                ("qc1", 384, 128, qT1, bqk1_sb, 0),
            )

            def qkproj(qb, items):
                for name, col0, m, dest, bias_sb, bcol in items:
                    for half in range(2):
                        qs = slice(qb * QB + half * 512,
                                   qb * QB + (half + 1) * 512)
                        ps = pjp.tile([128, 512], f32, name="pj", tag="pj")
                        for kt in range(NDT):
                            nc.tensor.matmul(
                                ps, wqkv_sb[kt][:, col0:col0 + 128],
                                xT_sb[kt][:, qs],
                                start=(kt == 0), stop=(kt == NDT - 1))
                        nc.vector.tensor_scalar_add(
                            dest[:, qs], ps[:m, :], bias_sb[:m, bcol:bcol + 1])

            # criticality order: what the first attention pass needs first
            qkproj(0, proj[0:1])          # kc0 qb0
            qkproj(0, proj[2:3])          # qc0 qb0
            qkproj(1, proj[0:1])          # kc0 qb1 (pass A covers all k)
            for st in range(NKT):
                ss = slice(st * 128, (st + 1) * 128)
                vps = pjp.tile([128, DC], f32, name="vps", tag="pj")
                for kt in range(NDT):
                    nc.tensor.matmul(
                        vps, xT_sb[kt][:, ss], wqkv_sb[kt][:, 512:512 + DC],
                        start=(kt == 0), stop=False)
                nc.tensor.matmul(vps, ones_row, bv_sb, start=False, stop=True)
                nc.vector.tensor_copy(
                    vv[st], vps.rearrange("p (h d) -> p h d", h=HPC))
            qkproj(0, proj[1:2])          # kc1 qb0
            qkproj(0, proj[3:4])          # qc1 qb0
            qkproj(1, proj[1:2])          # kc1 qb1
            qkproj(1, proj[2:3])          # qc0 qb1
            qkproj(1, proj[3:4])          # qc1 qb1

            # ---- phase 2 + 3 ----
            def denom_chain(qb, h, dn_slices):
                # dn_slices: list of (psum_row_ap, half) with the raw
                # denominator rows; move to partition 0, reciprocal,
                # broadcast into a per-head (or stacked) dbc tile slice.
                off = h * S + qb * QB
                stg = stagp.tile([128, QB], f32, name="stg", tag="stg")
                for row_ap, half in dn_slices:
                    nc.vector.tensor_copy(
                        stg[row_ap.base_partition():row_ap.base_partition() + 1,
                            half * 512:(half + 1) * 512], row_ap)
                    nc.sync.dma_start(
                        out=drow[0:1, off + half * 512: off + (half + 1) * 512],
                        in_=stg[row_ap.base_partition():
                                row_ap.base_partition() + 1,
                                half * 512:(half + 1) * 512])
                nc.vector.reciprocal_approx_fast(
                    drec[0:1, off:off + QB], drow[0:1, off:off + QB])

            def pass_ab(qb):
                qs = slice(qb * QB, (qb + 1) * QB)
                acc = acp.tile([128, QB], f32, name="accab", tag="ac")
                dn = [pjp.tile([128, 512], f32, name=f"dn{i}", tag="pj")
                      for i in range(2)]
                sc_t = {}
                at_t = {}

                def emit_scores(kt):
                    kslice = slice(kt * 128, (kt + 1) * 128)
                    for h in (0, 1):
                        sc_t[h] = scp.tile([128, QB], f32, name="sc", tag="sc")
                    for half in range(2):
                        hs = slice(qb * QB + half * 512,
                                   qb * QB + (half + 1) * 512)
                        for h in (0, 1):
                            hp = slice(h * HD, (h + 1) * HD)
                            nc.tensor.matmul(
                                sc_t[h][:, half * 512:(half + 1) * 512],
                                kT0[hp, kslice], qT0[hp, hs],
                                start=True, stop=True)

                emit_scores(0)
                for kt in range(NKT):
                    for h in (0, 1):
                        at = att.tile([128, QB], bf16, name="at", tag="at")
                        nc.scalar.activation(at, sc_t[h], Exp, scale=0.125)
                        at_t[h] = at
                    if kt + 1 < NKT:
                        emit_scores(kt + 1)
                    st, sp = (kt == 0), (kt == NKT - 1)
                    for half in range(2):
                        cs2 = slice(half * 512, (half + 1) * 512)
                        # col-packed quad: h0 rows 0:64, denominators at
                        # partitions 64/32 of dn, h1 rows 64:128
                        nc.tensor.matmul(acc[0:64, cs2], vv[kt][:, 0, :],
                                         at_t[0][:, cs2], start=st, stop=sp,
                                         skip_group_check=True)
                        nc.tensor.matmul(dn[half][64:65, :], ones_col,
                                         at_t[0][:, cs2], start=st, stop=sp,
                                         skip_group_check=True)
                        nc.tensor.matmul(dn[half][32:33, :], ones_col,
                                         at_t[1][:, cs2], start=st, stop=sp,
                                         skip_group_check=True)
                        nc.tensor.matmul(acc[64:128, cs2], vv[kt][:, 1, :],
                                         at_t[1][:, cs2], start=st, stop=sp,
                                         skip_group_check=True)
                # raw copy frees the accumulator, then normalize
                orw = orwp.tile([128, QB], bf16, name="orw", tag="orw")
                nc.vector.tensor_copy(orw, acc)
                denom_chain(qb, 0, [(dn[half][64:65, :], half)
                                    for half in range(2)])
                denom_chain(qb, 1, [(dn[half][32:33, :], half)
                                    for half in range(2)])
                dbc = dbcp.tile([128, QB], f32, name="dbc", tag="dbc")
                nc.gpsimd.partition_broadcast(
                    dbc[0:HD, :], drec[0:1, 0 * S + qb * QB:][:, :QB],
                    channels=HD)
                dbc1 = dbcp.tile([HD, QB], f32, name="dbc1", tag="dbc1")
                nc.gpsimd.partition_broadcast(
                    dbc1, drec[0:1, 1 * S + qb * QB:][:, :QB], channels=HD)
                nc.sync.dma_start(out=dbc[HD:128, :], in_=dbc1)
                nc.vector.tensor_mul(outT_ab[:, qs], orw, dbc)

            def pass_c(qb):
                qs = slice(qb * QB, (qb + 1) * QB)
                acc = acp.tile([128, QB], f32, name="accc", tag="ac")
                dn = [pjp.tile([128, 512], f32, name=f"dnc{i}", tag="pj")
                      for i in range(2)]
                sc_t = {}
                at_t = {}

                def emit_scores(ktp):
                    # kt pair: even kt on rows 0:64, odd on rows 64:128
                    for j in (0, 1):
                        kt = 2 * ktp + j
                        kslice = slice(kt * 128, (kt + 1) * 128)
                        hp = slice(j * HD, (j + 1) * HD)
                        sc = scp.tile([128, QB], f32, name="scc", tag="sc")
                        for half in range(2):
                            hs = slice(qb * QB + half * 512,
                                       qb * QB + (half + 1) * 512)
                            nc.tensor.matmul(
                                sc[:, half * 512:(half + 1) * 512],
                                kT1[hp, kslice], qT1[hp, hs],
                                start=True, stop=True)
                        sc_t[kt] = sc

                emit_scores(0)
                for ktp in range(NKT // 2):
                    for j in (0, 1):
                        kt = 2 * ktp + j
                        at = att.tile([128, QB], bf16, name="atc", tag="at")
                        nc.scalar.activation(at, sc_t[kt], Exp, scale=0.125)
                        at_t[kt] = at
                    if ktp + 1 < NKT // 2:
                        emit_scores(ktp + 1)
                    for j in (0, 1):
                        kt = 2 * ktp + j
                        st, sp = (kt == 0), (kt == NKT - 1)
                        for half in range(2):
                            cs2 = slice(half * 512, (half + 1) * 512)
                            nc.tensor.matmul(acc[0:64, cs2], vv[kt][:, 2, :],
                                             at_t[kt][:, cs2],
                                             start=st, stop=sp,
                                             skip_group_check=True)
                            nc.tensor.matmul(dn[half][64:65, :], ones_col,
                                             at_t[kt][:, cs2],
                                             start=st, stop=sp,
                                             skip_group_check=True)
                orw = orwp.tile([128, QB], bf16, name="orwc", tag="orw")
                nc.vector.tensor_copy(orw[0:HD, :], acc[0:HD, :])
                denom_chain(qb, 2, [(dn[half][64:65, :], half)
                                    for half in range(2)])
                dbc = dbcp.tile([128, QB], f32, name="dbcc", tag="dbc")
                off = 2 * S + qb * QB
                nc.gpsimd.partition_broadcast(
                    dbc[0:HD, :], drec[0:1, off:off + QB], channels=HD)
                nc.vector.tensor_mul(outT_c[:, qs], orw[0:HD, :],
                                     dbc[0:HD, :])

            acc_sbs = {}

            def out_block_ab(sub):
                # head 1 contribution (a lone row-64 matmul is legal; a
                # K-split accumulation across row positions is not)
                rs = slice(sub * 128, (sub + 1) * 128)
                acc_sb = accsb.tile([128, D], f32, name="acc_sb", tag="accsb")
                acc_sbs[sub] = acc_sb
                for c in range(2):
                    cs2 = slice(c * 384, (c + 1) * 384)
                    P = pjp.tile([128, 384], f32, name="P", tag="pj")
                    nc.tensor.matmul(P, outT_ab[HD:128, rs],
                                     wo_ab[HD:128, cs2],
                                     start=True, stop=True)
                    nc.vector.tensor_add(acc_sb[:, cs2], P, bo_sb[:, cs2])

            def out_block_c(sub):
                # heads 0 and 2: both at row position 0, one PSUM group
                rs = slice(sub * 128, (sub + 1) * 128)
                acc_sb = acc_sbs.pop(sub)
                for c in range(2):
                    cs2 = slice(c * 384, (c + 1) * 384)
                    P = pjp.tile([128, 384], f32, name="P2", tag="pj")
                    nc.tensor.matmul(P, outT_ab[0:HD, rs], wo_ab[0:HD, cs2],
                                     start=True, stop=False)
                    nc.tensor.matmul(P, outT_c[:, rs], wo_c2[:, cs2],
                                     start=False, stop=True)
                    nc.vector.tensor_add(acc_sb[:, cs2], P, acc_sb[:, cs2])
                nc.sync.dma_start(out=out.ap()[rs, :], in_=acc_sb)

            for qb in range(NQB):
                subs = range(qb * QB // 128, (qb + 1) * QB // 128)
                with tc.high_priority():
                    pass_ab(qb)
                for sub in subs:
                    out_block_ab(sub)
                with tc.high_priority():
                    pass_c(qb)
                for sub in subs:
                    out_block_c(sub)

    nc.compile()
    return nc


def _prep_core_inputs(x, Wq, bq, Wk, bk, Wv, bv, Wo, bo, core):
    b, g = divmod(core, 4)
    cs = slice(g * DC, (g + 1) * DC)
    xTb = np.ascontiguousarray(x[b].T).astype(BF16)
    Wq_c, Wk_c, Wv_c = Wq[:, cs], Wk[:, cs], Wv[:, cs]
    wqkv = np.concatenate(
        [Wk_c[:, :128], Wk_c[:, 128:], Wk_c[:, 128:],
         Wq_c[:, :128], Wq_c[:, 128:], Wq_c[:, 128:], Wv_c],
        axis=1).astype(BF16)
    wo_c = Wo[cs, :]  # (192, D): h0, h1, h2 row blocks
    bq_c, bk_c = bq[cs], bk[cs]
    bqk0 = np.stack([bq_c[:128], bk_c[:128]], axis=1).astype(np.float32)
    bqk1 = np.stack([np.tile(bq_c[128:], 2), np.tile(bk_c[128:], 2)],
                    axis=1).astype(np.float32)
    bo_t = (np.broadcast_to(bo, (128, D)) if g == 0
            else np.zeros((128, D), np.float32))
    return {
        "xT": xTb,
        "wqkv": np.ascontiguousarray(wqkv),
        "wo": np.ascontiguousarray(wo_c).astype(BF16),
        "bqk0": np.ascontiguousarray(bqk0),
        "bqk1": np.ascontiguousarray(bqk1),
        "bv": np.ascontiguousarray(bv[cs]).reshape(1, DC).astype(BF16),
        "bo_t": np.ascontiguousarray(bo_t).astype(np.float32),
    }


def kernel(x, Wq, bq, Wk, bk, Wv, bv, Wo, bo, _trace=False):
    from concourse.bass_utils import run_bass_kernel_spmd

    x = np.asarray(x, np.float32)
    Wq, bq = np.asarray(Wq, np.float32), np.asarray(bq, np.float32)
    Wk, bk = np.asarray(Wk, np.float32), np.asarray(bk, np.float32)
    Wv, bv = np.asarray(Wv, np.float32), np.asarray(bv, np.float32)
    Wo, bo = np.asarray(Wo, np.float32), np.asarray(bo, np.float32)

    if "nc" not in _cache:
        _cache["nc"] = _build_nc()
    nc = _cache["nc"]

    in_maps = [_prep_core_inputs(x, Wq, bq, Wk, bk, Wv, bv, Wo, bo, c)
               for c in range(8)]
    res = run_bass_kernel_spmd(nc, in_maps, core_ids=list(range(8)),
                               trace=_trace)
    _cache["last_result"] = res
    parts = [r["out"] for r in res.results]
    full = np.zeros((B, S, D), np.float32)
    for b in range(B):
        full[b] = parts[4 * b] + parts[4 * b + 1] + parts[4 * b + 2] + parts[4 * b + 3]
    return full


# revision 23
# speedup vs baseline: 1.3334x; 1.3334x over previous
"""Multi-head attention (B=2, S=2048, D=768, H=12) on 8 TRN2 NeuronCores.

Sharding: data-parallel over batch (2) x tensor-parallel over heads
(4 groups of 3 heads), Megatron-style. Core c handles batch c//4 and
heads 3*(c%4) .. 3*(c%4)+2. Each core computes a partial (S, D) output
(its heads' contribution through Wo); the host sums the 4 partials per
batch. bo is added on exactly one core per batch (the others get zeros).

Device kernel (per core), all matmuls bf16 with fp32 PSUM accumulation,
every matmul padded to M=128 output partitions (keeps FWL + PE activity
monitor engaged):
  phase 1: qT/kT (head-dim on partitions, zero-padded chunks for head 2)
           and v (natural layout, ones column at 64, zero-padded to 128
           for the softmax denominator) projected from xT = x[b].T.
  phase 2: per 1024-wide q block, per head: scoresT = k @ qT on PE
           (h0/h1 row-packed via K=64 tile positions), exp(scores/8) on
           ACT (PSUM->SBUF, bf16), outT = [v|1|0]^T @ attnT accumulated
           over 16 k-tiles in PSUM; row 64 of the accumulator is the
           softmax denominator. Denominators go: DVE copy (partition 64)
           -> SBUF->SBUF DMA to partition 0 -> DVE reciprocal -> GPSIMD
           partition_broadcast to a (64, 1024) tile -> the outT copy is
           a fused normalize (tensor_tensor mult).
  phase 3: per 128-row output block: P = sum_h outT_h.T @ Wo_h in one
           PSUM accumulation group, one DVE add of bo, DMA out. Shares
           the accumulator PSUM slots so it overlaps the next q block.
"""

import numpy as np
import ml_dtypes

BF16 = ml_dtypes.bfloat16

B, S, D = 2, 2048, 768
H, HD = 12, 64
HPC = 3            # heads per core
DC = HPC * HD      # 192 projection columns per core
NKT = S // 128     # 16 k-tiles
NDT = D // 128     # 6 contraction tiles for projections
QB = 1024          # q-block width for scores/exp
NQB = S // QB      # 2

_cache = {}


def _patch_ldw_opt():
    # walrus is invoked with --enable-ldw-opt=false by default; redundant
    # LDWEIGHTS elision is a large win for back-to-back matmul streams.
    import concourse.bass_utils as bu
    if getattr(bu, "_ldw_patched", False):
        return
    orig = bu.run_command

    def patched(argv, **kw):
        argv = [a
                for a in argv]
        return orig(argv, **kw)

    bu.run_command = patched
    bu._ldw_patched = True


def _build_nc():
    _patch_ldw_opt()
    import concourse.bacc as bacc
    import concourse.mybir as mybir
    import concourse.tile as tile

    f32 = mybir.dt.float32
    bf16 = mybir.dt.bfloat16
    Exp = mybir.ActivationFunctionType.Exp

    nc = bacc.Bacc("TRN2", target_bir_lowering=False, debug=False, num_devices=1)

    def mm(out_ap, lhsT, rhs, start, stop, nmax=512):
        n = rhs.shape[-1]
        for i in range(0, n, nmax):
            j = min(i + nmax, n)
            nc.tensor.matmul(out_ap[:, i:j], lhsT, rhs[:, i:j],
                             start=start, stop=stop)

    # wqkv columns: [q01 | q2+pad | k01 | k2+pad | v]
    xT = nc.dram_tensor("xT", (D, S), bf16, kind="ExternalInput")
    wqkv = nc.dram_tensor("wqkv", (D, 4 * 128 + DC), bf16, kind="ExternalInput")
    wo = nc.dram_tensor("wo", (HD, HPC, D), bf16, kind="ExternalInput")
    bqk0 = nc.dram_tensor("bqk0", (128, 2), f32, kind="ExternalInput")
    bqk1 = nc.dram_tensor("bqk1", (128, 2), f32, kind="ExternalInput")
    bv = nc.dram_tensor("bv", (1, DC), bf16, kind="ExternalInput")
    bo_t = nc.dram_tensor("bo_t", (128, D), f32, kind="ExternalInput")
    out = nc.dram_tensor("out", (S, D), f32, kind="ExternalOutput")

    with tile.TileContext(nc) as tc:
        with (
            tc.tile_pool(name="persist", bufs=1) as sbp,
            tc.tile_pool(name="att", bufs=6) as att,
            tc.tile_pool(name="stagp", bufs=2) as stagp,
            tc.tile_pool(name="dbcp", bufs=3) as dbcp,
            tc.tile_pool(name="orwp", bufs=4) as orwp,
            tc.tile_pool(name="accsb", bufs=9) as accsb,
            tc.tile_pool(name="scp", bufs=2, space="PSUM") as scp,
            tc.tile_pool(name="acp", bufs=1, space="PSUM") as acp,
            tc.tile_pool(name="pjp", bufs=2, space="PSUM") as pjp,
        ):
            # ---- persistent SBUF tensors + input DMAs ----
            xT_sb = []
            wqkv_sb = []
            for kt in range(NDT):
                ks = slice(kt * 128, (kt + 1) * 128)
                xt = sbp.tile([128, S], bf16, name=f"xT_sb{kt}")
                nc.sync.dma_start(out=xt, in_=xT.ap()[ks, :])
                xT_sb.append(xt)
                wt = sbp.tile([128, 4 * 128 + DC], bf16, name=f"wqkv_sb{kt}")
                nc.sync.dma_start(out=wt, in_=wqkv.ap()[ks, :])
                wqkv_sb.append(wt)
            wo_sb = sbp.tile([HD, HPC, D], bf16)
            nc.sync.dma_start(out=wo_sb, in_=wo.ap())
            bqk0_sb = sbp.tile([128, 2], f32)
            nc.sync.dma_start(out=bqk0_sb, in_=bqk0.ap())
            bqk1_sb = sbp.tile([128, 2], f32)
            nc.sync.dma_start(out=bqk1_sb, in_=bqk1.ap())
            bv_sb = sbp.tile([1, DC], bf16)
            nc.sync.dma_start(out=bv_sb, in_=bv.ap())
            bo_sb = sbp.tile([128, D], f32)
            nc.sync.dma_start(out=bo_sb, in_=bo_t.ap())

            ones_row = sbp.tile([1, 128], bf16)
            nc.vector.memset(ones_row, 1.0)
            ones_col = sbp.tile([128, 1], bf16)
            nc.vector.memset(ones_col, 1.0)

            # warm up the ACT exp table early (overlaps the input DMAs)
            wu = sbp.tile([1, 8], f32)
            nc.vector.memset(wu, 0.0)
            wu2 = sbp.tile([1, 8], f32)
            nc.scalar.activation(wu2, wu, Exp, scale=1.0)

            qT0 = sbp.tile([128, S], bf16)   # heads 0 (p0:64) / 1 (p64:128)
            kT0 = sbp.tile([128, S], bf16)
            qT1 = sbp.tile([128, S], bf16)   # head 2, duplicated on both halves
            kT1 = sbp.tile([128, S], bf16)
            # v natural: [v | ones | zeros] -> M=128 (denominator row 64)
            vv = [sbp.tile([128, HPC, 128], bf16, name=f"vv{st}")
                  for st in range(NKT)]
            outT = sbp.tile([HD, HPC, S], bf16)  # normalized outT, base 0
            drow = sbp.tile([1, HPC * S], f32)
            drec = sbp.tile([1, HPC * S], f32)

            # ---- phase 1: projections ----
            proj = (
                ("kc0", 0, 128, kT0, bqk0_sb, 1),
                ("kc1", 128, 128, kT1, bqk1_sb, 1),
                ("qc0", 256, 128, qT0, bqk0_sb, 0),
                ("qc1", 384, 128, qT1, bqk1_sb, 0),
            )

            def qkproj(qb, items):
                for name, col0, m, dest, bias_sb, bcol in items:
                    for half in range(2):
                        qs = slice(qb * QB + half * 512,
                                   qb * QB + (half + 1) * 512)
                        ps = pjp.tile([128, 512], f32, name="pj", tag="pj")
                        for kt in range(NDT):
                            nc.tensor.matmul(
                                ps, wqkv_sb[kt][:, col0:col0 + 128],
                                xT_sb[kt][:, qs],
                                start=(kt == 0), stop=(kt == NDT - 1))
                        nc.vector.tensor_scalar_add(
                            dest[:, qs], ps[:m, :], bias_sb[:m, bcol:bcol + 1])

            # criticality order: what the first attention pass needs first
            qkproj(0, proj[0:1])          # kc0 qb0
            qkproj(0, proj[2:3])          # qc0 qb0
            qkproj(1, proj[0:1])          # kc0 qb1 (pass A covers all k)
            for st in range(NKT):
                ss = slice(st * 128, (st + 1) * 128)
                vps = pjp.tile([128, DC], f32, name="vps", tag="pj")
                for kt in range(NDT):
                    nc.tensor.matmul(
                        vps, xT_sb[kt][:, ss], wqkv_sb[kt][:, 512:512 + DC],
                        start=(kt == 0), stop=False)
                nc.tensor.matmul(vps, ones_row, bv_sb, start=False, stop=True)
                nc.vector.tensor_copy(
                    vv[st][:, :, 0:HD],
                    vps.rearrange("p (h d) -> p h d", h=HPC))
                nc.vector.memset(vv[st][:, :, HD:HD + 1], 1.0)
                nc.vector.memset(vv[st][:, :, HD + 1:128], 0.0)
            qkproj(0, proj[1:2])          # kc1 qb0
            qkproj(0, proj[3:4])          # qc1 qb0
            qkproj(1, proj[1:2])          # kc1 qb1
            qkproj(1, proj[2:3])          # qc0 qb1
            qkproj(1, proj[3:4])          # qc1 qb1

            # ---- phase 2 + 3 ----
            def denom_chain(qb, h, dn_slices):
                # dn_slices: list of (psum_row_ap, half) with the raw
                # denominator rows; move to partition 0, reciprocal,
                # broadcast into a per-head (or stacked) dbc tile slice.
                off = h * S + qb * QB
                stg = stagp.tile([128, QB], f32, name="stg", tag="stg")
                for row_ap, half in dn_slices:
                    nc.vector.tensor_copy(
                        stg[row_ap.base_partition():row_ap.base_partition() + 1,
                            half * 512:(half + 1) * 512], row_ap)
                    nc.sync.dma_start(
                        out=drow[0:1, off + half * 512: off + (half + 1) * 512],
                        in_=stg[row_ap.base_partition():
                                row_ap.base_partition() + 1,
                                half * 512:(half + 1) * 512])
                nc.vector.reciprocal_approx_fast(
                    drec[0:1, off:off + QB], drow[0:1, off:off + QB])

            def attn_head_pass(qb, h):
                # one head: scoresT -> exp -> [v|1|0]^T @ attnT (M=128,
                # denominator lands on accumulator row 64)
                qs = slice(qb * QB, (qb + 1) * QB)
                acc = acp.tile([128, QB], f32, name="acc", tag="ac")
                sc_t = {}
                at_t = {}
                if h < 2:
                    qh, kh = qT0[h * HD:(h + 1) * HD, :], kT0[h * HD:(h + 1) * HD, :]
                    qo, ko = qh, kh

                def emit_scores(kt):
                    sc = scp.tile([128, QB], f32, name="sc", tag="sc")
                    if h == 2:
                        # head 2 is duplicated on both partition halves:
                        # alternate row position by kt so consecutive
                        # scores matmuls row-pack
                        j = kt % 2
                        hp = slice(j * HD, (j + 1) * HD)
                        kh_, qh_ = kT1[hp, :], qT1[hp, :]
                    else:
                        kh_, qh_ = kh, qh
                    for half in range(2):
                        hs = slice(qb * QB + half * 512,
                                   qb * QB + (half + 1) * 512)
                        nc.tensor.matmul(sc[:, half * 512:(half + 1) * 512],
                                         kh_[:, kt * 128:(kt + 1) * 128],
                                         qh_[:, hs], start=True, stop=True)
                    sc_t[kt] = sc

                emit_scores(0)
                for kt in range(NKT):
                    at = att.tile([128, QB], bf16, name="at", tag="at")
                    nc.scalar.activation(at, sc_t.pop(kt), Exp, scale=0.125)
                    at_t[kt] = at
                    if kt + 1 < NKT:
                        emit_scores(kt + 1)
                    mm(acc, vv[kt][:, h, :], at_t.pop(kt),
                       start=(kt == 0), stop=(kt == NKT - 1))
                # release accumulator fast with raw copies, normalize later
                orw = orwp.tile([HD, QB], bf16, name="orw", tag="orw")
                nc.vector.tensor_copy(orw, acc[0:HD, :])
                off = h * S + qb * QB
                stg = stagp.tile([128, QB], f32, name="stg", tag="stg")
                nc.vector.tensor_copy(stg[64:65, :], acc[64:65, :])
                nc.sync.dma_start(out=drow[0:1, off:off + QB],
                                  in_=stg[64:65, :])
                nc.vector.reciprocal_approx_fast(
                    drec[0:1, off:off + QB], drow[0:1, off:off + QB])
                dbc = dbcp.tile([HD, QB], f32, name="dbc", tag="dbc")
                nc.gpsimd.partition_broadcast(
                    dbc, drec[0:1, off:off + QB], channels=HD)
                nc.vector.tensor_mul(outT[:, h, qs], orw, dbc)

            acc_sbs = {}

            def out_block_ab(sub):
                # heads 0/1 contribution (both at row position 0)
                rs = slice(sub * 128, (sub + 1) * 128)
                acc_sb = accsb.tile([128, D], f32, name="acc_sb", tag="accsb")
                acc_sbs[sub] = acc_sb
                for c in range(2):
                    cs2 = slice(c * 384, (c + 1) * 384)
                    P = pjp.tile([128, 384], f32, name="P", tag="pj")
                    nc.tensor.matmul(P, outT[:, 0, rs], wo_sb[:, 0, cs2],
                                     start=True, stop=False)
                    nc.tensor.matmul(P, outT[:, 1, rs], wo_sb[:, 1, cs2],
                                     start=False, stop=True)
                    nc.vector.tensor_add(acc_sb[:, cs2], P, bo_sb[:, cs2])

            def out_block_c(sub):
                rs = slice(sub * 128, (sub + 1) * 128)
                acc_sb = acc_sbs.pop(sub)
                for c in range(2):
                    cs2 = slice(c * 384, (c + 1) * 384)
                    P = pjp.tile([128, 384], f32, name="P2", tag="pj")
                    nc.tensor.matmul(P, outT[:, 2, rs], wo_sb[:, 2, cs2],
                                     start=True, stop=True)
                    nc.vector.tensor_add(acc_sb[:, cs2], P, acc_sb[:, cs2])
                nc.sync.dma_start(out=out.ap()[rs, :], in_=acc_sb)

            for qb in range(NQB):
                subs = range(qb * QB // 128, (qb + 1) * QB // 128)
                with tc.high_priority():
                    attn_head_pass(qb, 0)
                    attn_head_pass(qb, 1)
                for sub in subs:
                    out_block_ab(sub)
                with tc.high_priority():
                    attn_head_pass(qb, 2)
                for sub in subs:
                    out_block_c(sub)

    nc.compile()
    return nc


def _prep_core_inputs(x, Wq, bq, Wk, bk, Wv, bv, Wo, bo, core):
    b, g = divmod(core, 4)
    cs = slice(g * DC, (g + 1) * DC)
    xTb = np.ascontiguousarray(x[b].T).astype(BF16)
    Wq_c, Wk_c, Wv_c = Wq[:, cs], Wk[:, cs], Wv[:, cs]
    wqkv = np.concatenate(
        [Wk_c[:, :128], Wk_c[:, 128:], Wk_c[:, 128:],
         Wq_c[:, :128], Wq_c[:, 128:], Wq_c[:, 128:], Wv_c],
        axis=1).astype(BF16)
    wo_c = Wo[cs, :].reshape(HPC, HD, D).transpose(1, 0, 2)  # (HD, HPC, D)
    bq_c, bk_c = bq[cs], bk[cs]
    bqk0 = np.stack([bq_c[:128], bk_c[:128]], axis=1).astype(np.float32)
    bqk1 = np.stack([np.tile(bq_c[128:], 2), np.tile(bk_c[128:], 2)],
                    axis=1).astype(np.float32)
    bo_t = (np.broadcast_to(bo, (128, D)) if g == 0
            else np.zeros((128, D), np.float32))
    return {
        "xT": xTb,
        "wqkv": np.ascontiguousarray(wqkv),
        "wo": np.ascontiguousarray(wo_c).astype(BF16),
        "bqk0": np.ascontiguousarray(bqk0),
        "bqk1": np.ascontiguousarray(bqk1),
        "bv": np.ascontiguousarray(bv[cs]).reshape(1, DC).astype(BF16),
        "bo_t": np.ascontiguousarray(bo_t).astype(np.float32),
    }


def kernel(x, Wq, bq, Wk, bk, Wv, bv, Wo, bo, _trace=False):
    from concourse.bass_utils import run_bass_kernel_spmd

    x = np.asarray(x, np.float32)
    Wq, bq = np.asarray(Wq, np.float32), np.asarray(bq, np.float32)
    Wk, bk = np.asarray(Wk, np.float32), np.asarray(bk, np.float32)
    Wv, bv = np.asarray(Wv, np.float32), np.asarray(bv, np.float32)
    Wo, bo = np.asarray(Wo, np.float32), np.asarray(bo, np.float32)

    if "nc" not in _cache:
        _cache["nc"] = _build_nc()
    nc = _cache["nc"]

    in_maps = [_prep_core_inputs(x, Wq, bq, Wk, bk, Wv, bv, Wo, bo, c)
               for c in range(8)]
    res = run_bass_kernel_spmd(nc, in_maps, core_ids=list(range(8)),
                               trace=_trace)
    _cache["last_result"] = res
    parts = [r["out"] for r in res.results]
    full = np.zeros((B, S, D), np.float32)
    for b in range(B):
        full[b] = parts[4 * b] + parts[4 * b + 1] + parts[4 * b + 2] + parts[4 * b + 3]
    return full
